# revision 10
# baseline (speedup 1.0000x reference)
"""Trainium2 Bass kernel for BatchNorm2d + 8-head self-attention block.

Reference (per batch element b, all fp32):
    xn = BN_eval(x[b]); t = xn.T
    q/k/v = t @ W.T + b            # [S, 512], 8 heads x 64
    attn  = softmax(q k^T / 8)     # per head
    y[b]  = ((attn v) @ wo.T + bo).T

Sharding: pure data parallel - one batch element per NeuronCore, weights
replicated, no collectives.

The kernel is paced by the ScalarE exp (64 ACTIVATEs x ~1.11us = 71us of
engine time; every other engine has slack).  Design goals, in order:
  1. ACT back-to-back: a flat 64-step pipeline emits scores(i+1) + exp(i+1)
     BEFORE pv(i), so the next exp's input is ready the moment the current
     exp finishes, and pv (which waits on exp in the PE's strict FIFO) never
     delays the next scores.
  2. Short lead-in: x / wv / wq+wk(hp0,1) DMAs are prioritized in first-need
     order; PE warmup covers the DMA wait; V projections and remaining
     QK/O projections are deadline-scheduled fillers inside the pipeline.
  3. Short tail: last head-pair normalizes straight out of PSUM, the final
     out-proj is a single K-accumulated group per column pair, y is fp16.

Per-step PSUM budget (8 banks): scores 2 tiles x [128,1024]f32 (4 banks),
po accumulators 2 x [65,512] (2), projection accumulators 2 x [128,512] (2).
po banks are freed ~0.7us after the last PV by a full-tile DVE copy; the
softmax normalize (recip + gpsimd partition-broadcast + mul) runs on the
SBUF copy off the critical path.

Matmul dtype fp16: 2-byte weights keep LDWEIGHTS in the PE background
buffer; ~1e-4 rel error end to end.
"""

import numpy as np

import concourse.bass as bass
import concourse.tile as tile
from concourse import bacc, mybir
from concourse.bass_utils import run_bass_kernel_spmd
from concourse.tile import add_dep_helper

B, C, S = 8, 512, 1024
H, DH, INNER = 8, 64, 512
EPS = 1e-5
SCALE = DH ** (-0.5)
N_CORES = 8
F32 = mybir.dt.float32
F16 = mybir.dt.float16

DT_MM = F16

_CACHE: dict = {}

KC = C // 128      # 4 contraction chunks over channels
IT = INNER // 128  # 4 tiles over inner dim (also head-pair index)
ST = S // 128      # 8 t-chunks
NSLAB = S // 512   # 2 s-slabs

N_WARM = 22        # PE warmup matmuls (N=256) covering the initial DMA wait
FILLER_CAP = 2     # optional filler matmuls per pipeline step


def build_bass(dt_mm):
    nc = bacc.Bacc("TRN2", target_bir_lowering=False, debug=False,
                   num_devices=N_CORES)

    # Host pre-arranges into SBUF layouts; every DMA is contiguous.
    x_d = nc.dram_tensor("x", [128, NSLAB, KC, 512], dt_mm, kind="ExternalInput")
    wqk_d = nc.dram_tensor("wqkT", [128, 2, IT, KC, 128], dt_mm, kind="ExternalInput")
    wv_d = nc.dram_tensor("wvT", [128, KC, 512], dt_mm, kind="ExternalInput")
    wo_d = nc.dram_tensor("woT", [128, KC, 512], dt_mm, kind="ExternalInput")
    # bq | bk | bo packed on host as [128, 12] (col t+0/4/8 = vec[t*128+p])
    bias_d = nc.dram_tensor("bias_pack", [128, 3 * IT], F32, kind="ExternalInput")
    y_d = nc.dram_tensor("y", [C, S], F16, kind="ExternalOutput")

    with tile.TileContext(nc) as tc:
        with (
            tc.tile_pool(name="persist", bufs=1) as persist,
            tc.tile_pool(name="out", bufs=2) as outp,
            tc.tile_pool(name="et", bufs=4) as etp,
            tc.tile_pool(name="norm", bufs=2) as normp,
            tc.tile_pool(name="psS", bufs=2, space="PSUM") as psS,
            tc.tile_pool(name="psPO", bufs=2, space="PSUM") as psPO,
            tc.tile_pool(name="psPR", bufs=2, space="PSUM") as psPR,
        ):
            # ---- persistent SBUF ----
            xr = persist.tile([128, NSLAB, KC, 512], dt_mm, tag="xr", name="xr")
            wqkr = persist.tile([128, 2, IT, KC, 128], dt_mm, tag="wqkr", name="wqkr")
            wqr = wqkr[:, 0]
            wkr = wqkr[:, 1]
            wvr = persist.tile([128, KC, 512], dt_mm, tag="wvr", name="wvr")
            wor = persist.tile([128, KC, 512], dt_mm, tag="wor", name="wor")

            # ---- loads, chained on the sync/HWDGE queue in need-order ----
            dmas = []
            dmas.append(nc.sync.dma_start(xr[:, 0], x_d[:, 0]))            # x slab0
            dmas.append(nc.sync.dma_start(wqkr[:, :, 0:1], wqk_d[:, :, 0:1]))  # q+k hp0
            dmas.append(nc.sync.dma_start(wvr[:], wv_d[:]))                # wv
            dmas.append(nc.sync.dma_start(wqkr[:, :, 1:2], wqk_d[:, :, 1:2]))  # q+k hp1
            dmas.append(nc.sync.dma_start(xr[:, 1], x_d[:, 1]))            # x slab1
            dmas.append(nc.sync.dma_start(wqkr[:, :, 2:4], wqk_d[:, :, 2:4]))  # q+k hp2,3
            dmas.append(nc.sync.dma_start(wor[:], wo_d[:]))
            for a, b in zip(dmas[1:], dmas):
                add_dep_helper(a.ins, b.ins, sync=False, reason="dma priority")

            # ---- PE warmup first: its memset heads the DVE queue, so the
            # PE is busy (HAM warming) during the whole initial DMA wait ----
            warm_sb = persist.tile([128, 256], dt_mm, tag="warm", name="warm_sb")
            nc.vector.memset(warm_sb[:], 0.0)
            warm_ps = psPR.tile([128, 512], F32, tag="acc", name="warm_ps")
            for wi in range(N_WARM):
                nc.tensor.matmul(warm_ps[:, 0:256], warm_sb[:, 0:128],
                                 warm_sb[:], start=(wi == 0),
                                 stop=(wi == N_WARM - 1))

            bias_sb = persist.tile([128, 3 * IT], F32, tag="bias")
            nc.gpsimd.dma_start(bias_sb[:], bias_d[:])
            bq_sb = bias_sb[:, 0:IT]
            bk_sb = bias_sb[:, IT:2 * IT]
            bo_sb = bias_sb[:, 2 * IT:3 * IT]

            # ---- persistent per-slab outputs ----
            qT = [[persist.tile([128, 512], dt_mm, tag=f"qT{i}{s}",
                                name=f"qT{i}{s}") for s in range(NSLAB)]
                  for i in range(IT)]
            kT = [[persist.tile([128, 512], dt_mm, tag=f"kT{i}{s}",
                                name=f"kT{i}{s}") for s in range(NSLAB)]
                  for i in range(IT)]
            oT = [[persist.tile([128, 512], dt_mm, tag=f"oT{i}{s}",
                                name=f"oT{i}{s}") for s in range(NSLAB)]
                  for i in range(IT)]
            v_sb = [persist.tile([128, H * 65], dt_mm, tag=f"v{t}",
                                 name=f"v{t}") for t in range(ST)]
            # ones columns of each v tile: memset the whole tile once; the
            # per-chunk evacuations overwrite the 64 data columns per head
            for t in range(ST):
                nc.vector.memset(v_sb[t][:], 1.0)
            # y slab1 partial (ic 0..2 of out-proj + bias), [p, ct, s]
            y_part = persist.tile([128, IT, 512], F32, tag="yp", name="y_part")

            # ---------- thunk builders (each thunk = one matmul; the last
            # in a group appends the PSUM evacuation) ----------
            def group_thunks(n_mm, emit_mm, evac):
                box = []

                def mk(i):
                    def t():
                        if i == 0:
                            box.append(psPR.tile([128, 512], F32,
                                                 tag="acc", name="acc"))
                        emit_mm(box[0], i)
                        if i == n_mm - 1:
                            evac(box[0])
                    return t

                return [mk(i) for i in range(n_mm)]

            def qk_thunks(w, bias, dst, hp, sl):
                def emit_mm(ps, kc):
                    nc.tensor.matmul(
                        ps[:], w[:, hp, kc, :], xr[:, sl, kc, :],
                        start=(kc == 0), stop=(kc == KC - 1),
                    )

                def evac(ps):
                    nc.vector.tensor_scalar_add(
                        dst[hp][sl][:], ps[:], bias[:, hp:hp + 1]
                    )

                return group_thunks(KC, emit_mm, evac)

            def v_thunks(tc_):
                def emit_mm(ps, kc):
                    nc.tensor.matmul(
                        ps[:], xr[:, tc_ // 4, kc, (tc_ % 4) * 128:(tc_ % 4 + 1) * 128],
                        wvr[:, kc, :],
                        start=(kc == 0), stop=(kc == KC - 1),
                    )

                def evac(ps):
                    vv = v_sb[tc_][:].rearrange("p (h m) -> p h m", h=H)
                    nc.vector.tensor_copy(
                        vv[:, :, 0:64], ps[:].rearrange("p (h m) -> p h m", h=H)
                    )

                return group_thunks(KC, emit_mm, evac)

            def op_thunks(ct):
                # full out-proj for slab 0, column tile ct (+bias, dma out)
                def emit_mm(ps, ic):
                    nc.tensor.matmul(
                        ps[:], wor[:, ic, ct * 128:(ct + 1) * 128], oT[ic][0][:],
                        start=(ic == 0), stop=(ic == IT - 1),
                    )

                def evac(ps):
                    ysb = outp.tile([128, 512], F16, tag="ysb", name="ysb")
                    nc.vector.tensor_scalar_add(ysb[:], ps[:],
                                                bo_sb[:, ct:ct + 1])
                    nc.sync.dma_start(
                        y_d[ct * 128:(ct + 1) * 128, 0:512], ysb[:]
                    )

                return group_thunks(IT, emit_mm, evac)

            def op_partial_a_thunks(ct):
                # ic 0,1 of the slab-1 out-proj, banked into SBUF (+bias)
                def emit_mm(ps, ic):
                    nc.tensor.matmul(
                        ps[:], wor[:, ic, ct * 128:(ct + 1) * 128], oT[ic][1][:],
                        start=(ic == 0), stop=(ic == 1),
                    )

                def evac(ps):
                    nc.vector.tensor_scalar_add(y_part[:, ct, :], ps[:],
                                                bo_sb[:, ct:ct + 1])

                return group_thunks(2, emit_mm, evac)

            def op_partial_b_thunks(ct):
                # ic 2, accumulated into y_part
                def emit_mm(ps, _):
                    nc.tensor.matmul(
                        ps[:], wor[:, 2, ct * 128:(ct + 1) * 128], oT[2][1][:],
                        start=True, stop=True,
                    )

                def evac(ps):
                    nc.vector.tensor_add(y_part[:, ct, :], y_part[:, ct, :],
                                         ps[:])

                return group_thunks(1, emit_mm, evac)

            # ---------- the flat attention pipeline ----------
            SLHP = [(sl, hp) for sl in range(NSLAB) for hp in range(IT)]
            NSTEP = len(SLHP) * ST  # 64

            def emit_scores(gi):
                sidx, tc_ = gi // ST, gi % ST
                sl, hp = SLHP[sidx]
                ksl, kcol = tc_ // 4, (tc_ % 4) * 128
                pss = psS.tile([128, 1024], F32, tag="pss", name=f"pss{gi}")
                nc.tensor.matmul(
                    pss[:, 0:512], kT[hp][ksl][0:64, kcol:kcol + 128],
                    qT[hp][sl][0:64, :],
                    start=True, stop=True, tile_position=(0, 0),
                )
                nc.tensor.matmul(
                    pss[:, 512:1024], kT[hp][ksl][64:128, kcol:kcol + 128],
                    qT[hp][sl][64:128, :],
                    start=True, stop=True, tile_position=(64, 0),
                )
                et = etp.tile([128, 1024], dt_mm, tag="et", name=f"et{gi}")
                nc.scalar.activation(
                    et[:], pss[:], mybir.ActivationFunctionType.Exp
                )
                return et

            po_cur = [None, None]

            def emit_pv(gi, et):
                sidx, tc_ = gi // ST, gi % ST
                sl, hp = SLHP[sidx]
                if tc_ == 0:
                    po_cur[0] = psPO.tile([65, 512], F32, tag="po", name=f"po0_{sidx}")
                    po_cur[1] = psPO.tile([65, 512], F32, tag="po", name=f"po1_{sidx}")
                h0, h1 = 2 * hp, 2 * hp + 1
                nc.tensor.matmul(
                    po_cur[0][:], v_sb[tc_][:, h0 * 65:(h0 + 1) * 65],
                    et[:, 0:512], start=(tc_ == 0), stop=(tc_ == ST - 1),
                )
                nc.tensor.matmul(
                    po_cur[1][:], v_sb[tc_][:, h1 * 65:(h1 + 1) * 65],
                    et[:, 512:1024], start=(tc_ == 0), stop=(tc_ == ST - 1),
                )

            def emit_norm(sidx, last):
                # NB: partition bases must be 32-aligned; the custom
                # reciprocal and the gpsimd broadcast both silently corrupt
                # on inputs not at partition 0 — so the den row is copied to
                # a [1,512] tile first.  Ops are interleaved across the two
                # halves so the gpsimd broadcasts run back to back.
                sl, hp = SLHP[sidx]
                srcs, rbcs = [], []
                for half in range(2):
                    po = po_cur[half]
                    drow = normp.tile([1, 512], F32, tag="drow", name="drow")
                    nc.vector.tensor_copy(drow[:], po[64:65, :])
                    if last:
                        srcs.append(po[0:64, :])
                    else:
                        # copying the data rows out frees the PSUM bank for
                        # the next head-pair ~1.4us after the last PV
                        u = normp.tile([64, 512], F32, tag="u", name=f"u{sidx}{half}")
                        nc.vector.tensor_copy(u[:], po[0:64, :])
                        srcs.append(u[:])
                    rrow = normp.tile([1, 512], F32, tag="rrow", name="rrow")
                    nc.vector.reciprocal_approx_fast(rrow[:], drow[:])
                    rbc = normp.tile([64, 512], F32, tag="rbc", name="rbc")
                    nc.gpsimd.partition_broadcast(rbc[:], rrow[:])
                    rbcs.append(rbc)
                for half in range(2):
                    nc.vector.tensor_mul(
                        oT[hp][sl][half * 64:(half + 1) * 64, :],
                        srcs[half], rbcs[half][:],
                    )

            # ---------- filler schedule: (ready, deadline, thunk) ----------
            # deadline d = must be emitted during step d at the latest
            # (fillers pop before that step's PV).
            fillers = []

            def push(ready, deadline, thunks):
                for t in thunks:
                    fillers.append([ready, deadline, t])

            # V projections: v_sb[tc] consumed by PV at step tc.  QK groups
            # get staggered deadlines comfortably before their consumers
            # (scores for step 8*hp+s4 are emitted one step early) so no
            # single step carries a lump of forced pops.
            for t in range(1, ST):
                push(0, t, v_thunks(t))
            push(0, 1, qk_thunks(wkr, bk_sb, kT, 0, 1))         # d0 (<=2)
            push(0, 4, qk_thunks(wqr, bq_sb, qT, 1, 0))         # a1 (<=6)
            push(0, 5, qk_thunks(wkr, bk_sb, kT, 1, 0))         # c1 (<=6)
            push(0, 9, qk_thunks(wkr, bk_sb, kT, 1, 1))         # d1 (<=10)
            push(0, 11, qk_thunks(wqr, bq_sb, qT, 2, 0))        # a2 (<=14)
            push(0, 12, qk_thunks(wkr, bk_sb, kT, 2, 0))        # c2 (<=14)
            push(0, 17, qk_thunks(wkr, bk_sb, kT, 2, 1))        # d2 (<=18)
            push(0, 19, qk_thunks(wqr, bq_sb, qT, 3, 0))        # a3 (<=22)
            push(0, 20, qk_thunks(wkr, bk_sb, kT, 3, 0))        # c3 (<=22)
            push(0, 25, qk_thunks(wkr, bk_sb, kT, 3, 1))        # d3 (<=26)
            push(0, 28, qk_thunks(wqr, bq_sb, qT, 0, 1))        # b0 (<=30)
            push(0, 34, qk_thunks(wqr, bq_sb, qT, 1, 1))        # b1 (<=38)
            push(0, 42, qk_thunks(wqr, bq_sb, qT, 2, 1))        # b2 (<=46)
            push(0, 50, qk_thunks(wqr, bq_sb, qT, 3, 1))        # b3 (<=54)
            # out-proj slab0: oT[*][0] complete ~2 steps after the step-31
            # normalize is emitted.
            for ct in range(IT):
                push(34, (37, 41, 45, 49)[ct], op_thunks(ct))
            # out-proj slab1 partials: ic 0,1 land after the step-47
            # normalize, ic 2 after step-55.
            for ct in range(IT):
                push(50, 51 + ct, op_partial_a_thunks(ct))
            for ct in range(IT):
                push(58, 58 + ct, op_partial_b_thunks(ct))

            fillers.sort(key=lambda f: f[1])

            fq = list(fillers)

            def pop_fillers(gi):
                n = 0
                cap = 0 if gi % ST == ST - 1 else FILLER_CAP
                while fq and fq[0][0] <= gi and (fq[0][1] <= gi or n < cap):
                    fq.pop(0)[2]()
                    n += 1

            # ---------- emission ----------
            run = lambda ts: [t() for t in ts]
            run(qk_thunks(wqr, bq_sb, qT, 0, 0))
            run(qk_thunks(wkr, bk_sb, kT, 0, 0))
            et_cur = emit_scores(0)
            run(v_thunks(0))
            for gi in range(NSTEP):
                et_next = emit_scores(gi + 1) if gi + 1 < NSTEP else None
                pop_fillers(gi)
                emit_pv(gi, et_cur)
                if gi % ST == ST - 1:
                    emit_norm(gi // ST, last=(gi == NSTEP - 1))
                et_cur = et_next
            assert not fq, f"{len(fq)} fillers left unscheduled"

            # keep the PE warm through the final normalize chain so the
            # fin matmuls don't run at the cold HAM clock
            for wi in range(24):
                nc.tensor.matmul(warm_ps[:, 0:256], warm_sb[:, 0:128],
                                 warm_sb[:], start=(wi == 0), stop=(wi == 23))

            # ---------- tail: final out-proj (ic=3) for slab 1 ----------
            # two [128,1024] psum tiles from the (now idle) scores pool
            fins = [psS.tile([128, 1024], F32, tag="pss", name=f"fin{pair}")
                    for pair in range(2)]
            for half in range(2):
                for ct in range(IT):
                    nc.tensor.matmul(
                        fins[ct // 2][:, (ct % 2) * 512:(ct % 2 + 1) * 512],
                        wor[half * 64:(half + 1) * 64, IT - 1,
                            ct * 128:(ct + 1) * 128],
                        oT[IT - 1][1][half * 64:(half + 1) * 64, :],
                        start=(half == 0), stop=(half == 1),
                        tile_position=(64 * half, 0),
                    )
            for ct in range(IT):
                fin = fins[ct // 2]
                j = ct % 2
                ysb = outp.tile([128, 512], F16, tag="ysbf", name="ysbf",
                                bufs=4)
                nc.vector.tensor_add(ysb[:], y_part[:, ct, :],
                                     fin[:, j * 512:(j + 1) * 512])
                eng = nc.sync if ct % 2 == 0 else nc.gpsimd
                eng.dma_start(y_d[ct * 128:(ct + 1) * 128, 512:1024], ysb[:])

    nc.compile()
    return nc


def prep_host(inputs, dt_mm):
    """Fold BN + scale + v-bias into effective weights (fp32 numpy)."""
    x = np.asarray(inputs["x"], dtype=np.float32)
    g = np.asarray(inputs["bn_gamma"], dtype=np.float32)
    be = np.asarray(inputs["bn_beta"], dtype=np.float32)
    mu = np.asarray(inputs["bn_mean"], dtype=np.float32)
    var = np.asarray(inputs["bn_var"], dtype=np.float32)
    wq = np.asarray(inputs["wq"], dtype=np.float32)
    bq = np.asarray(inputs["bq"], dtype=np.float32)
    wk = np.asarray(inputs["wk"], dtype=np.float32)
    bk = np.asarray(inputs["bk"], dtype=np.float32)
    wv = np.asarray(inputs["wv"], dtype=np.float32)
    bv = np.asarray(inputs["bv"], dtype=np.float32)
    wo = np.asarray(inputs["wo"], dtype=np.float32)
    bo = np.asarray(inputs["bo"], dtype=np.float32)

    a = g / np.sqrt(var + EPS)          # [C]
    bvec = be - mu * a                  # [C]

    wq_eff = wq * a[None, :] * SCALE
    bq_eff = (bq + wq @ bvec) * SCALE
    wk_eff = wk * a[None, :]
    bk_eff = bk + wk @ bvec
    wv_eff = wv * a[None, :]
    bv_eff = bv + wv @ bvec
    bo_eff = bo + wo @ bv_eff           # v bias rides through softmax (sums to 1)

    bias_pack = np.concatenate(
        [bq_eff.reshape(IT, 128).T, bk_eff.reshape(IT, 128).T,
         bo_eff.reshape(IT, 128).T], axis=1
    ).astype(np.float32)

    np_dt = np.float16

    def dev_layout(a):
        # [C_or_I, N] -> [128, KC, N]: partition p holds rows {k*128+p}
        return np.ascontiguousarray(
            a.reshape(KC, 128, a.shape[1]).transpose(1, 0, 2).astype(np_dt))

    def dev_layout_hp(a):
        # [C, I] -> [128, IT, KC, 128]: [p, i, k, c] = a[k*128+p, i*128+c]
        return np.ascontiguousarray(
            a.reshape(KC, 128, IT, 128).transpose(1, 2, 0, 3).astype(np_dt))

    def dev_layout_x(a):
        # [C, S] -> [128, NSLAB, KC, 512]: [p, s, k, c] = a[k*128+p, s*512+c]
        return np.ascontiguousarray(
            a.reshape(KC, 128, NSLAB, 512).transpose(1, 2, 0, 3).astype(np_dt))

    wqk_l = np.ascontiguousarray(np.stack(
        [dev_layout_hp(wq_eff.T), dev_layout_hp(wk_eff.T)], axis=1))
    wv_l = dev_layout(wv_eff.T)
    wo_l = dev_layout(wo.T)
    per_core = []
    for b in range(B):
        per_core.append({
            "x": dev_layout_x(x[b, :, :, 0]),
            "wqkT": wqk_l,
            "wvT": wv_l,
            "woT": wo_l,
            "bias_pack": np.ascontiguousarray(bias_pack),
        })
    return per_core


def _get_nc(dt_mm):
    key = str(dt_mm)
    if key not in _CACHE:
        _CACHE[key] = build_bass(dt_mm)
    return _CACHE[key]


def kernel(**inputs):
    nc = _get_nc(DT_MM)
    in_maps = prep_host(inputs, DT_MM)
    res = run_bass_kernel_spmd(nc, in_maps, list(range(N_CORES)))
    y = np.stack([res.results[c]["y"].astype(np.float32)
                  for c in range(N_CORES)], axis=0)
    return y[..., None]


def run_traced(**inputs):
    """Like kernel() but with NTFF profiling; returns (y, results, tmpdir)."""
    nc = _get_nc(DT_MM)
    in_maps = prep_host(inputs, DT_MM)
    import tempfile
    tmpdir = tempfile.mkdtemp(prefix="mha_trace_")
    res = run_bass_kernel_spmd(
        nc, in_maps, list(range(N_CORES)), trace=True, tmpdir=tmpdir
    )
    y = np.stack([res.results[c]["y"].astype(np.float32)
                  for c in range(N_CORES)], axis=0)
    return y[..., None], res, tmpdir


# revision 15
# speedup vs baseline: 1.0154x; 1.0154x over previous
"""Trainium2 Bass kernel for BatchNorm2d + 8-head self-attention block.

Reference (per batch element b, all fp32):
    xn = BN_eval(x[b]); t = xn.T
    q/k/v = t @ W.T + b            # [S, 512], 8 heads x 64
    attn  = softmax(q k^T / 8)     # per head
    y[b]  = ((attn v) @ wo.T + bo).T

Sharding: pure data parallel - one batch element per NeuronCore, weights
replicated, no collectives.

The kernel is paced by the ScalarE exp (64 ACTIVATEs x ~1.11us = 71us of
engine time; every other engine has slack).  Design goals, in order:
  1. ACT back-to-back: a flat 64-step pipeline emits scores(i+1) + exp(i+1)
     BEFORE pv(i), so the next exp's input is ready the moment the current
     exp finishes, and pv (which waits on exp in the PE's strict FIFO) never
     delays the next scores.
  2. Short lead-in: x / wv / wq+wk(hp0,1) DMAs are prioritized in first-need
     order; PE warmup covers the DMA wait; V projections and remaining
     QK/O projections are deadline-scheduled fillers inside the pipeline.
  3. Short tail: last head-pair normalizes straight out of PSUM, the final
     out-proj is a single K-accumulated group per column pair, y is fp16.

Per-step PSUM budget (8 banks): scores 2 tiles x [128,1024]f32 (4 banks),
po accumulators 2 x [65,512] (2), projection accumulators 2 x [128,512] (2).
po banks are freed ~0.7us after the last PV by a full-tile DVE copy; the
softmax normalize (recip + gpsimd partition-broadcast + mul) runs on the
SBUF copy off the critical path.

Matmul dtype fp16: 2-byte weights keep LDWEIGHTS in the PE background
buffer; ~1e-4 rel error end to end.
"""

import numpy as np

import concourse.bass as bass
import concourse.tile as tile
from concourse import bacc, mybir
from concourse.bass_utils import run_bass_kernel_spmd
from concourse.tile import add_dep_helper

B, C, S = 8, 512, 1024
H, DH, INNER = 8, 64, 512
EPS = 1e-5
SCALE = DH ** (-0.5)
N_CORES = 8
F32 = mybir.dt.float32
F16 = mybir.dt.float16

DT_MM = F16

_CACHE: dict = {}

KC = C // 128      # 4 contraction chunks over channels
IT = INNER // 128  # 4 tiles over inner dim (also head-pair index)
ST = S // 128      # 8 t-chunks
NSLAB = S // 512   # 2 s-slabs

N_WARM = 22        # PE warmup matmuls (N=256) covering the initial DMA wait
FILLER_CAP = 2     # optional filler matmuls per pipeline step


def build_bass(dt_mm):
    nc = bacc.Bacc("TRN2", target_bir_lowering=False, debug=False,
                   num_devices=N_CORES)

    # Host pre-arranges into SBUF layouts; every DMA is contiguous.
    x_d = nc.dram_tensor("x", [128, NSLAB, KC, 512], dt_mm, kind="ExternalInput")
    wqk_d = nc.dram_tensor("wqkT", [128, 2, IT, KC, 128], dt_mm, kind="ExternalInput")
    wv_d = nc.dram_tensor("wvT", [128, KC, 512], dt_mm, kind="ExternalInput")
    wo_d = nc.dram_tensor("woT", [128, KC, 512], dt_mm, kind="ExternalInput")
    # bq | bk | bo packed on host as [128, 12] (col t+0/4/8 = vec[t*128+p])
    bias_d = nc.dram_tensor("bias_pack", [128, 3 * IT], F32, kind="ExternalInput")
    y_d = nc.dram_tensor("y", [C, S], F16, kind="ExternalOutput")

    with tile.TileContext(nc) as tc:
        with (
            tc.tile_pool(name="persist", bufs=1) as persist,
            tc.tile_pool(name="out", bufs=2) as outp,
            tc.tile_pool(name="et", bufs=4) as etp,
            tc.tile_pool(name="norm", bufs=2) as normp,
            tc.tile_pool(name="psS", bufs=2, space="PSUM") as psS,
            tc.tile_pool(name="psPO", bufs=2, space="PSUM") as psPO,
            tc.tile_pool(name="psPR", bufs=2, space="PSUM") as psPR,
        ):
            # ---- persistent SBUF ----
            xr = persist.tile([128, NSLAB, KC, 512], dt_mm, tag="xr", name="xr")
            wqkr = persist.tile([128, 2, IT, KC, 128], dt_mm, tag="wqkr", name="wqkr")
            wqr = wqkr[:, 0]
            wkr = wqkr[:, 1]
            wvr = persist.tile([128, KC, 512], dt_mm, tag="wvr", name="wvr")
            wor = persist.tile([128, KC, 512], dt_mm, tag="wor", name="wor")

            # ---- loads, chained on the sync/HWDGE queue in need-order ----
            dmas = []
            dmas.append(nc.sync.dma_start(xr[:, 0], x_d[:, 0]))            # x slab0
            dmas.append(nc.sync.dma_start(wvr[:], wv_d[:]))                # wv
            dmas.append(nc.sync.dma_start(wqkr[:, :, 0:1], wqk_d[:, :, 0:1]))  # q+k hp0
            dmas.append(nc.sync.dma_start(wqkr[:, :, 1:2], wqk_d[:, :, 1:2]))  # q+k hp1
            dmas.append(nc.sync.dma_start(xr[:, 1], x_d[:, 1]))            # x slab1
            dmas.append(nc.sync.dma_start(wqkr[:, :, 2:4], wqk_d[:, :, 2:4]))  # q+k hp2,3
            dmas.append(nc.sync.dma_start(wor[:], wo_d[:]))
            for a, b in zip(dmas[1:], dmas):
                add_dep_helper(a.ins, b.ins, sync=False, reason="dma priority")

            # ---- PE warmup first: its memset heads the DVE queue, so the
            # PE is busy (HAM warming) during the whole initial DMA wait ----
            warm_sb = persist.tile([128, 256], dt_mm, tag="warm", name="warm_sb")
            nc.vector.memset(warm_sb[:], 0.0)
            warm_ps = psS.tile([128, 1024], F32, tag="pss", name="warm_ps")
            for wi in range(N_WARM):
                nc.tensor.matmul(warm_ps[:, 0:256], warm_sb[:, 0:128],
                                 warm_sb[:], start=(wi == 0),
                                 stop=(wi == N_WARM - 1))

            bias_sb = persist.tile([128, 3 * IT], F32, tag="bias")
            nc.gpsimd.dma_start(bias_sb[:], bias_d[:])
            bq_sb = bias_sb[:, 0:IT]
            bk_sb = bias_sb[:, IT:2 * IT]
            bo_sb = bias_sb[:, 2 * IT:3 * IT]

            # ---- persistent per-slab outputs ----
            qT = [[persist.tile([128, 512], dt_mm, tag=f"qT{i}{s}",
                                name=f"qT{i}{s}") for s in range(NSLAB)]
                  for i in range(IT)]
            kT = [[persist.tile([128, 512], dt_mm, tag=f"kT{i}{s}",
                                name=f"kT{i}{s}") for s in range(NSLAB)]
                  for i in range(IT)]
            oT = [[persist.tile([128, 512], dt_mm, tag=f"oT{i}{s}",
                                name=f"oT{i}{s}") for s in range(NSLAB)]
                  for i in range(IT)]
            v_sb = [persist.tile([128, H * 65], dt_mm, tag=f"v{t}",
                                 name=f"v{t}") for t in range(ST)]
            # ones columns of each v tile: memset the whole tile once; the
            # per-chunk evacuations overwrite the 64 data columns per head
            for t in range(ST):
                nc.vector.memset(v_sb[t][:], 1.0)
            # y slab1 partial (ic 0..2 of out-proj + bias), [p, ct, s]
            y_part = persist.tile([128, IT, 512], F32, tag="yp", name="y_part")

            # ---------- thunk builders (each thunk = one matmul; the last
            # in a group appends the PSUM evacuation) ----------
            def group_thunks(n_mm, emit_mm, evac):
                box = []

                def mk(i):
                    def t():
                        if i == 0:
                            box.append(psPR.tile([128, 512], F32,
                                                 tag="acc", name="acc"))
                        emit_mm(box[0], i)
                        if i == n_mm - 1:
                            evac(box[0])
                    return t

                return [mk(i) for i in range(n_mm)]

            def qk_thunks(w, bias, dst, hp, sl):
                def emit_mm(ps, kc):
                    nc.tensor.matmul(
                        ps[:], w[:, hp, kc, :], xr[:, sl, kc, :],
                        start=(kc == 0), stop=(kc == KC - 1),
                    )

                def evac(ps):
                    nc.vector.tensor_scalar_add(
                        dst[hp][sl][:], ps[:], bias[:, hp:hp + 1]
                    )

                return group_thunks(KC, emit_mm, evac)

            def v_thunks(tc_):
                def emit_mm(ps, kc):
                    nc.tensor.matmul(
                        ps[:], xr[:, tc_ // 4, kc, (tc_ % 4) * 128:(tc_ % 4 + 1) * 128],
                        wvr[:, kc, :],
                        start=(kc == 0), stop=(kc == KC - 1),
                    )

                def evac(ps):
                    vv = v_sb[tc_][:].rearrange("p (h m) -> p h m", h=H)
                    nc.vector.tensor_copy(
                        vv[:, :, 0:64], ps[:].rearrange("p (h m) -> p h m", h=H)
                    )

                return group_thunks(KC, emit_mm, evac)

            def op_thunks(ct):
                # full out-proj for slab 0, column tile ct (+bias, dma out)
                def emit_mm(ps, ic):
                    nc.tensor.matmul(
                        ps[:], wor[:, ic, ct * 128:(ct + 1) * 128], oT[ic][0][:],
                        start=(ic == 0), stop=(ic == IT - 1),
                    )

                def evac(ps):
                    ysb = outp.tile([128, 512], F16, tag="ysb", name="ysb")
                    nc.vector.tensor_scalar_add(ysb[:], ps[:],
                                                bo_sb[:, ct:ct + 1])
                    nc.sync.dma_start(
                        y_d[ct * 128:(ct + 1) * 128, 0:512], ysb[:]
                    )

                return group_thunks(IT, emit_mm, evac)

            def op_partial_a_thunks(ct):
                # ic 0,1 of the slab-1 out-proj, banked into SBUF (+bias)
                def emit_mm(ps, ic):
                    nc.tensor.matmul(
                        ps[:], wor[:, ic, ct * 128:(ct + 1) * 128], oT[ic][1][:],
                        start=(ic == 0), stop=(ic == 1),
                    )

                def evac(ps):
                    nc.vector.tensor_scalar_add(y_part[:, ct, :], ps[:],
                                                bo_sb[:, ct:ct + 1])

                return group_thunks(2, emit_mm, evac)

            def op_partial_b_thunks(ct):
                # ic 2, accumulated into y_part
                def emit_mm(ps, _):
                    nc.tensor.matmul(
                        ps[:], wor[:, 2, ct * 128:(ct + 1) * 128], oT[2][1][:],
                        start=True, stop=True,
                    )

                def evac(ps):
                    nc.vector.tensor_add(y_part[:, ct, :], y_part[:, ct, :],
                                         ps[:])

                return group_thunks(1, emit_mm, evac)

            # ---------- the flat attention pipeline ----------
            SLHP = [(sl, hp) for sl in range(NSLAB) for hp in range(IT)]
            NSTEP = len(SLHP) * ST  # 64

            def emit_scores(gi):
                sidx, tc_ = gi // ST, gi % ST
                sl, hp = SLHP[sidx]
                ksl, kcol = tc_ // 4, (tc_ % 4) * 128
                pss = psS.tile([128, 1024], F32, tag="pss", name=f"pss{gi}")
                nc.tensor.matmul(
                    pss[:, 0:512], kT[hp][ksl][0:64, kcol:kcol + 128],
                    qT[hp][sl][0:64, :],
                    start=True, stop=True, tile_position=(0, 0),
                )
                nc.tensor.matmul(
                    pss[:, 512:1024], kT[hp][ksl][64:128, kcol:kcol + 128],
                    qT[hp][sl][64:128, :],
                    start=True, stop=True, tile_position=(64, 0),
                )
                et = etp.tile([128, 1024], dt_mm, tag="et", name=f"et{gi}")
                nc.scalar.activation(
                    et[:], pss[:], mybir.ActivationFunctionType.Exp
                )
                return et

            po_cur = [None, None]

            def emit_pv(gi, et):
                sidx, tc_ = gi // ST, gi % ST
                sl, hp = SLHP[sidx]
                if tc_ == 0:
                    po_cur[0] = psPO.tile([65, 512], F32, tag="po", name=f"po0_{sidx}")
                    po_cur[1] = psPO.tile([65, 512], F32, tag="po", name=f"po1_{sidx}")
                h0, h1 = 2 * hp, 2 * hp + 1
                nc.tensor.matmul(
                    po_cur[0][:], v_sb[tc_][:, h0 * 65:(h0 + 1) * 65],
                    et[:, 0:512], start=(tc_ == 0), stop=(tc_ == ST - 1),
                )
                nc.tensor.matmul(
                    po_cur[1][:], v_sb[tc_][:, h1 * 65:(h1 + 1) * 65],
                    et[:, 512:1024], start=(tc_ == 0), stop=(tc_ == ST - 1),
                )

            def emit_norm(sidx, last):
                # NB: partition bases must be 32-aligned; the custom
                # reciprocal and the gpsimd broadcast both silently corrupt
                # on inputs not at partition 0 — so the den row is copied to
                # a [1,512] tile first.  Ops are interleaved across the two
                # halves so the gpsimd broadcasts run back to back.
                sl, hp = SLHP[sidx]
                srcs, rbcs = [], []
                for half in range(2):
                    po = po_cur[half]
                    drow = normp.tile([1, 512], F32, tag="drow", name="drow")
                    nc.vector.tensor_copy(drow[:], po[64:65, :])
                    if half == 0:
                        emit_norm.last_drow = drow
                    if last:
                        srcs.append(po[0:64, :])
                    else:
                        # copying the data rows out frees the PSUM bank for
                        # the next head-pair ~1.4us after the last PV
                        u = normp.tile([64, 512], F32, tag="u", name=f"u{sidx}{half}")
                        nc.vector.tensor_copy(u[:], po[0:64, :])
                        srcs.append(u[:])
                    rrow = normp.tile([1, 512], F32, tag="rrow", name="rrow")
                    nc.vector.reciprocal_approx_fast(rrow[:], drow[:])
                    rbc = normp.tile([64, 512], F32, tag="rbc", name="rbc")
                    nc.gpsimd.partition_broadcast(rbc[:], rrow[:])
                    rbcs.append(rbc)
                for half in range(2):
                    nc.vector.tensor_mul(
                        oT[hp][sl][half * 64:(half + 1) * 64, :],
                        srcs[half], rbcs[half][:],
                    )

            # ---------- filler schedule: (ready, deadline, thunk) ----------
            # deadline d = must be emitted during step d at the latest
            # (fillers pop before that step's PV).
            fillers = []

            def push(ready, deadline, thunks):
                for t in thunks:
                    fillers.append([ready, deadline, t])

            # V projections: v_sb[tc] consumed by PV at step tc.  QK groups
            # get staggered deadlines comfortably before their consumers
            # (scores for step 8*hp+s4 are emitted one step early) so no
            # single step carries a lump of forced pops.
            for t in range(4, ST):
                push(0, t, v_thunks(t))
            push(0, 1, qk_thunks(wkr, bk_sb, kT, 0, 1))         # d0 (<=2)
            push(0, 4, qk_thunks(wqr, bq_sb, qT, 1, 0))         # a1 (<=6)
            push(0, 5, qk_thunks(wkr, bk_sb, kT, 1, 0))         # c1 (<=6)
            push(0, 9, qk_thunks(wkr, bk_sb, kT, 1, 1))         # d1 (<=10)
            push(0, 11, qk_thunks(wqr, bq_sb, qT, 2, 0))        # a2 (<=14)
            push(0, 12, qk_thunks(wkr, bk_sb, kT, 2, 0))        # c2 (<=14)
            push(0, 17, qk_thunks(wkr, bk_sb, kT, 2, 1))        # d2 (<=18)
            push(0, 19, qk_thunks(wqr, bq_sb, qT, 3, 0))        # a3 (<=22)
            push(0, 20, qk_thunks(wkr, bk_sb, kT, 3, 0))        # c3 (<=22)
            push(0, 25, qk_thunks(wkr, bk_sb, kT, 3, 1))        # d3 (<=26)
            push(0, 28, qk_thunks(wqr, bq_sb, qT, 0, 1))        # b0 (<=30)
            push(0, 34, qk_thunks(wqr, bq_sb, qT, 1, 1))        # b1 (<=38)
            push(0, 42, qk_thunks(wqr, bq_sb, qT, 2, 1))        # b2 (<=46)
            push(0, 50, qk_thunks(wqr, bq_sb, qT, 3, 1))        # b3 (<=54)
            # out-proj slab0: oT[*][0] complete ~2 steps after the step-31
            # normalize is emitted.
            for ct in range(IT):
                push(34, (37, 41, 45, 49)[ct], op_thunks(ct))
            # out-proj slab1 partials: ic 0,1 land after the step-47
            # normalize, ic 2 after step-55.
            for ct in range(IT):
                push(50, 51 + ct, op_partial_a_thunks(ct))
            for ct in range(IT):
                push(58, 58 + ct, op_partial_b_thunks(ct))

            fillers.sort(key=lambda f: f[1])

            fq = list(fillers)

            def pop_fillers(gi):
                n = 0
                cap = 0 if gi % ST == ST - 1 else FILLER_CAP
                while fq and fq[0][0] <= gi and (fq[0][1] <= gi or n < cap):
                    fq.pop(0)[2]()
                    n += 1

            # ---------- emission ----------
            run = lambda ts: [t() for t in ts]
            for t in range(4):
                run(v_thunks(t))
            run(qk_thunks(wqr, bq_sb, qT, 0, 0))
            run(qk_thunks(wkr, bk_sb, kT, 0, 0))
            et_cur = emit_scores(0)
            for gi in range(NSTEP):
                et_next = emit_scores(gi + 1) if gi + 1 < NSTEP else None
                pop_fillers(gi)
                emit_pv(gi, et_cur)
                if gi % ST == ST - 1:
                    emit_norm(gi // ST, last=(gi == NSTEP - 1))
                et_cur = et_next
            assert not fq, f"{len(fq)} fillers left unscheduled"


            # ---------- tail: final out-proj (ic=3) for slab 1 ----------
            # two [128,1024] psum tiles from the (now idle) scores pool
            fins = [psS.tile([128, 1024], F32, tag="pss", name=f"fin{pair}")
                    for pair in range(2)]
            for half in range(2):
                for ct in range(IT):
                    nc.tensor.matmul(
                        fins[ct // 2][:, (ct % 2) * 512:(ct % 2 + 1) * 512],
                        wor[half * 64:(half + 1) * 64, IT - 1,
                            ct * 128:(ct + 1) * 128],
                        oT[IT - 1][1][half * 64:(half + 1) * 64, :],
                        start=(half == 0), stop=(half == 1),
                        tile_position=(64 * half, 0),
                    )
            for ct in range(IT):
                fin = fins[ct // 2]
                j = ct % 2
                ysb = outp.tile([128, 512], F16, tag="ysbf", name="ysbf",
                                bufs=4)
                nc.vector.tensor_add(ysb[:], y_part[:, ct, :],
                                     fin[:, j * 512:(j + 1) * 512])
                eng = nc.sync if ct % 2 == 0 else nc.gpsimd
                eng.dma_start(y_d[ct * 128:(ct + 1) * 128, 512:1024], ysb[:])

    nc.compile()
    return nc


def prep_host(inputs, dt_mm):
    """Fold BN + scale + v-bias into effective weights (fp32 numpy)."""
    x = np.asarray(inputs["x"], dtype=np.float32)
    g = np.asarray(inputs["bn_gamma"], dtype=np.float32)
    be = np.asarray(inputs["bn_beta"], dtype=np.float32)
    mu = np.asarray(inputs["bn_mean"], dtype=np.float32)
    var = np.asarray(inputs["bn_var"], dtype=np.float32)
    wq = np.asarray(inputs["wq"], dtype=np.float32)
    bq = np.asarray(inputs["bq"], dtype=np.float32)
    wk = np.asarray(inputs["wk"], dtype=np.float32)
    bk = np.asarray(inputs["bk"], dtype=np.float32)
    wv = np.asarray(inputs["wv"], dtype=np.float32)
    bv = np.asarray(inputs["bv"], dtype=np.float32)
    wo = np.asarray(inputs["wo"], dtype=np.float32)
    bo = np.asarray(inputs["bo"], dtype=np.float32)

    a = g / np.sqrt(var + EPS)          # [C]
    bvec = be - mu * a                  # [C]

    wq_eff = wq * a[None, :] * SCALE
    bq_eff = (bq + wq @ bvec) * SCALE
    wk_eff = wk * a[None, :]
    bk_eff = bk + wk @ bvec
    wv_eff = wv * a[None, :]
    bv_eff = bv + wv @ bvec
    bo_eff = bo + wo @ bv_eff           # v bias rides through softmax (sums to 1)

    bias_pack = np.concatenate(
        [bq_eff.reshape(IT, 128).T, bk_eff.reshape(IT, 128).T,
         bo_eff.reshape(IT, 128).T], axis=1
    ).astype(np.float32)

    np_dt = np.float16

    def dev_layout(a):
        # [C_or_I, N] -> [128, KC, N]: partition p holds rows {k*128+p}
        return np.ascontiguousarray(
            a.reshape(KC, 128, a.shape[1]).transpose(1, 0, 2).astype(np_dt))

    def dev_layout_hp(a):
        # [C, I] -> [128, IT, KC, 128]: [p, i, k, c] = a[k*128+p, i*128+c]
        return np.ascontiguousarray(
            a.reshape(KC, 128, IT, 128).transpose(1, 2, 0, 3).astype(np_dt))

    def dev_layout_x(a):
        # [C, S] -> [128, NSLAB, KC, 512]: [p, s, k, c] = a[k*128+p, s*512+c]
        return np.ascontiguousarray(
            a.reshape(KC, 128, NSLAB, 512).transpose(1, 2, 0, 3).astype(np_dt))

    wqk_l = np.ascontiguousarray(np.stack(
        [dev_layout_hp(wq_eff.T), dev_layout_hp(wk_eff.T)], axis=1))
    wv_l = dev_layout(wv_eff.T)
    wo_l = dev_layout(wo.T)
    per_core = []
    for b in range(B):
        per_core.append({
            "x": dev_layout_x(x[b, :, :, 0]),
            "wqkT": wqk_l,
            "wvT": wv_l,
            "woT": wo_l,
            "bias_pack": np.ascontiguousarray(bias_pack),
        })
    return per_core


def _get_nc(dt_mm):
    key = str(dt_mm)
    if key not in _CACHE:
        _CACHE[key] = build_bass(dt_mm)
    return _CACHE[key]


def kernel(**inputs):
    nc = _get_nc(DT_MM)
    in_maps = prep_host(inputs, DT_MM)
    res = run_bass_kernel_spmd(nc, in_maps, list(range(N_CORES)))
    y = np.stack([res.results[c]["y"].astype(np.float32)
                  for c in range(N_CORES)], axis=0)
    return y[..., None]


def run_traced(**inputs):
    """Like kernel() but with NTFF profiling; returns (y, results, tmpdir)."""
    nc = _get_nc(DT_MM)
    in_maps = prep_host(inputs, DT_MM)
    import tempfile
    tmpdir = tempfile.mkdtemp(prefix="mha_trace_")
    res = run_bass_kernel_spmd(
        nc, in_maps, list(range(N_CORES)), trace=True, tmpdir=tmpdir
    )
    y = np.stack([res.results[c]["y"].astype(np.float32)
                  for c in range(N_CORES)], axis=0)
    return y[..., None], res, tmpdir


# revision 17
# speedup vs baseline: 1.0251x; 1.0096x over previous
"""Trainium2 Bass kernel for BatchNorm2d + 8-head self-attention block.

Reference (per batch element b, all fp32):
    xn = BN_eval(x[b]); t = xn.T
    q/k/v = t @ W.T + b            # [S, 512], 8 heads x 64
    attn  = softmax(q k^T / 8)     # per head
    y[b]  = ((attn v) @ wo.T + bo).T

Sharding: pure data parallel - one batch element per NeuronCore, weights
replicated, no collectives.

The kernel is paced by the ScalarE exp (64 ACTIVATEs x ~1.11us = 71us of
engine time; every other engine has slack).  Design goals, in order:
  1. ACT back-to-back: a flat 64-step pipeline emits scores(i+1) + exp(i+1)
     BEFORE pv(i), so the next exp's input is ready the moment the current
     exp finishes, and pv (which waits on exp in the PE's strict FIFO) never
     delays the next scores.
  2. Short lead-in: x / wv / wq+wk(hp0,1) DMAs are prioritized in first-need
     order; PE warmup covers the DMA wait; V projections and remaining
     QK/O projections are deadline-scheduled fillers inside the pipeline.
  3. Short tail: last head-pair normalizes straight out of PSUM, the final
     out-proj is a single K-accumulated group per column pair, y is fp16.

Per-step PSUM budget (8 banks): scores 2 tiles x [128,1024]f32 (4 banks),
po accumulators 2 x [65,512] (2), projection accumulators 2 x [128,512] (2).
po banks are freed ~0.7us after the last PV by a full-tile DVE copy; the
softmax normalize (recip + gpsimd partition-broadcast + mul) runs on the
SBUF copy off the critical path.

Matmul dtype fp16: 2-byte weights keep LDWEIGHTS in the PE background
buffer; ~1e-4 rel error end to end.
"""

import numpy as np

import concourse.bass as bass
import concourse.tile as tile
from concourse import bacc, mybir
from concourse.bass_utils import run_bass_kernel_spmd
from concourse.tile import add_dep_helper

B, C, S = 8, 512, 1024
H, DH, INNER = 8, 64, 512
EPS = 1e-5
SCALE = DH ** (-0.5)
N_CORES = 8
F32 = mybir.dt.float32
F16 = mybir.dt.float16

DT_MM = F16

_CACHE: dict = {}

KC = C // 128      # 4 contraction chunks over channels
IT = INNER // 128  # 4 tiles over inner dim (also head-pair index)
ST = S // 128      # 8 t-chunks
NSLAB = S // 512   # 2 s-slabs

N_WARM = 22        # PE warmup matmuls (N=256) covering the initial DMA wait
FILLER_CAP = 2     # optional filler matmuls per pipeline step


def build_bass(dt_mm):
    nc = bacc.Bacc("TRN2", target_bir_lowering=False, debug=False,
                   num_devices=N_CORES)

    # Host pre-arranges into SBUF layouts; every DMA is contiguous.
    x_d = nc.dram_tensor("x", [128, NSLAB, KC, 512], dt_mm, kind="ExternalInput")
    wqk_d = nc.dram_tensor("wqkT", [128, 2, IT, KC, 128], dt_mm, kind="ExternalInput")
    wv_d = nc.dram_tensor("wvT", [128, KC, 512], dt_mm, kind="ExternalInput")
    wo_d = nc.dram_tensor("woT", [128, KC, 512], dt_mm, kind="ExternalInput")
    # bq | bk | bo packed on host as [128, 12] (col t+0/4/8 = vec[t*128+p])
    bias_d = nc.dram_tensor("bias_pack", [128, 3 * IT], F32, kind="ExternalInput")
    y_d = nc.dram_tensor("y", [C, S], F16, kind="ExternalOutput")

    with tile.TileContext(nc) as tc:
        with (
            tc.tile_pool(name="persist", bufs=1) as persist,
            tc.tile_pool(name="out", bufs=2) as outp,
            tc.tile_pool(name="et", bufs=4) as etp,
            tc.tile_pool(name="norm", bufs=2) as normp,
            tc.tile_pool(name="psS", bufs=2, space="PSUM") as psS,
            tc.tile_pool(name="psPO", bufs=2, space="PSUM") as psPO,
            tc.tile_pool(name="psPR", bufs=2, space="PSUM") as psPR,
        ):
            # ---- persistent SBUF ----
            xr = persist.tile([128, NSLAB, KC, 512], dt_mm, tag="xr", name="xr")
            wqkr = persist.tile([128, 2, IT, KC, 128], dt_mm, tag="wqkr", name="wqkr")
            wqr = wqkr[:, 0]
            wkr = wqkr[:, 1]
            wvr = persist.tile([128, KC, 512], dt_mm, tag="wvr", name="wvr")
            wor = persist.tile([128, KC, 512], dt_mm, tag="wor", name="wor")

            # ---- loads, chained on the sync/HWDGE queue in need-order ----
            # critical prefix on three queues in parallel; the rest chained
            # on the sync queue behind x slab0
            nc.sync.dma_start(xr[:, 0], x_d[:, 0])                         # x slab0
            nc.scalar.dma_start(wvr[:], wv_d[:])                           # wv
            nc.gpsimd.dma_start(wqkr[:, :, 0:2], wqk_d[:, :, 0:2])         # q+k hp0,1
            dmas = []
            dmas.append(nc.sync.dma_start(xr[:, 1], x_d[:, 1]))            # x slab1
            dmas.append(nc.sync.dma_start(wqkr[:, :, 2:4], wqk_d[:, :, 2:4]))  # q+k hp2,3
            dmas.append(nc.sync.dma_start(wor[:], wo_d[:]))
            for a, b in zip(dmas[1:], dmas):
                add_dep_helper(a.ins, b.ins, sync=False, reason="dma priority")

            # ---- PE warmup first: its memset heads the DVE queue, so the
            # PE is busy (HAM warming) during the whole initial DMA wait ----
            warm_sb = persist.tile([128, 256], dt_mm, tag="warm", name="warm_sb")
            nc.vector.memset(warm_sb[:], 0.0)
            warm_ps = psS.tile([128, 1024], F32, tag="pss", name="warm_ps")
            for wi in range(N_WARM):
                nc.tensor.matmul(warm_ps[:, 0:256], warm_sb[:, 0:128],
                                 warm_sb[:], start=(wi == 0),
                                 stop=(wi == N_WARM - 1))

            bias_sb = persist.tile([128, 3 * IT], F32, tag="bias")
            nc.gpsimd.dma_start(bias_sb[:], bias_d[:])
            bq_sb = bias_sb[:, 0:IT]
            bk_sb = bias_sb[:, IT:2 * IT]
            bo_sb = bias_sb[:, 2 * IT:3 * IT]

            # ---- persistent per-slab outputs ----
            qT = [[persist.tile([128, 512], dt_mm, tag=f"qT{i}{s}",
                                name=f"qT{i}{s}") for s in range(NSLAB)]
                  for i in range(IT)]
            kT = [[persist.tile([128, 512], dt_mm, tag=f"kT{i}{s}",
                                name=f"kT{i}{s}") for s in range(NSLAB)]
                  for i in range(IT)]
            oT = [[persist.tile([128, 512], dt_mm, tag=f"oT{i}{s}",
                                name=f"oT{i}{s}") for s in range(NSLAB)]
                  for i in range(IT)]
            v_sb = [persist.tile([128, H * 65], dt_mm, tag=f"v{t}",
                                 name=f"v{t}") for t in range(ST)]
            # ones columns of each v tile: memset the whole tile once; the
            # per-chunk evacuations overwrite the 64 data columns per head
            for t in range(ST):
                nc.vector.memset(v_sb[t][:], 1.0)
            # y slab1 partial (ic 0..2 of out-proj + bias), [p, ct, s]
            y_part = persist.tile([128, IT, 512], F32, tag="yp", name="y_part")

            # ---------- thunk builders (each thunk = one matmul; the last
            # in a group appends the PSUM evacuation) ----------
            def group_thunks(n_mm, emit_mm, evac):
                box = []

                def mk(i):
                    def t():
                        if i == 0:
                            box.append(psPR.tile([128, 512], F32,
                                                 tag="acc", name="acc"))
                        emit_mm(box[0], i)
                        if i == n_mm - 1:
                            evac(box[0])
                    return t

                return [mk(i) for i in range(n_mm)]

            def qk_thunks(w, bias, dst, hp, sl):
                def emit_mm(ps, kc):
                    nc.tensor.matmul(
                        ps[:], w[:, hp, kc, :], xr[:, sl, kc, :],
                        start=(kc == 0), stop=(kc == KC - 1),
                    )

                def evac(ps):
                    nc.vector.tensor_scalar_add(
                        dst[hp][sl][:], ps[:], bias[:, hp:hp + 1]
                    )

                return group_thunks(KC, emit_mm, evac)

            def v_thunks(tc_):
                def emit_mm(ps, kc):
                    nc.tensor.matmul(
                        ps[:], xr[:, tc_ // 4, kc, (tc_ % 4) * 128:(tc_ % 4 + 1) * 128],
                        wvr[:, kc, :],
                        start=(kc == 0), stop=(kc == KC - 1),
                    )

                def evac(ps):
                    vv = v_sb[tc_][:].rearrange("p (h m) -> p h m", h=H)
                    nc.vector.tensor_copy(
                        vv[:, :, 0:64], ps[:].rearrange("p (h m) -> p h m", h=H)
                    )

                return group_thunks(KC, emit_mm, evac)

            def op_thunks(ct):
                # full out-proj for slab 0, column tile ct (+bias, dma out)
                def emit_mm(ps, ic):
                    nc.tensor.matmul(
                        ps[:], wor[:, ic, ct * 128:(ct + 1) * 128], oT[ic][0][:],
                        start=(ic == 0), stop=(ic == IT - 1),
                    )

                def evac(ps):
                    ysb = outp.tile([128, 512], F16, tag="ysb", name="ysb")
                    nc.vector.tensor_scalar_add(ysb[:], ps[:],
                                                bo_sb[:, ct:ct + 1])
                    nc.sync.dma_start(
                        y_d[ct * 128:(ct + 1) * 128, 0:512], ysb[:]
                    )

                return group_thunks(IT, emit_mm, evac)

            def op_partial_a_thunks(ct):
                # ic 0,1 of the slab-1 out-proj, banked into SBUF (+bias)
                def emit_mm(ps, ic):
                    nc.tensor.matmul(
                        ps[:], wor[:, ic, ct * 128:(ct + 1) * 128], oT[ic][1][:],
                        start=(ic == 0), stop=(ic == 1),
                    )

                def evac(ps):
                    nc.vector.tensor_scalar_add(y_part[:, ct, :], ps[:],
                                                bo_sb[:, ct:ct + 1])

                return group_thunks(2, emit_mm, evac)

            def op_partial_b_thunks(ct):
                # ic 2, accumulated into y_part
                def emit_mm(ps, _):
                    nc.tensor.matmul(
                        ps[:], wor[:, 2, ct * 128:(ct + 1) * 128], oT[2][1][:],
                        start=True, stop=True,
                    )

                def evac(ps):
                    nc.vector.tensor_add(y_part[:, ct, :], y_part[:, ct, :],
                                         ps[:])

                return group_thunks(1, emit_mm, evac)

            # ---------- the flat attention pipeline ----------
            SLHP = [(sl, hp) for sl in range(NSLAB) for hp in range(IT)]
            NSTEP = len(SLHP) * ST  # 64

            def emit_scores(gi):
                sidx, tc_ = gi // ST, gi % ST
                sl, hp = SLHP[sidx]
                ksl, kcol = tc_ // 4, (tc_ % 4) * 128
                pss = psS.tile([128, 1024], F32, tag="pss", name=f"pss{gi}")
                nc.tensor.matmul(
                    pss[:, 0:512], kT[hp][ksl][0:64, kcol:kcol + 128],
                    qT[hp][sl][0:64, :],
                    start=True, stop=True, tile_position=(0, 0),
                )
                nc.tensor.matmul(
                    pss[:, 512:1024], kT[hp][ksl][64:128, kcol:kcol + 128],
                    qT[hp][sl][64:128, :],
                    start=True, stop=True, tile_position=(64, 0),
                )
                et = etp.tile([128, 1024], dt_mm, tag="et", name=f"et{gi}")
                nc.scalar.activation(
                    et[:], pss[:], mybir.ActivationFunctionType.Exp
                )
                return et

            po_cur = [None, None]

            def emit_pv(gi, et):
                sidx, tc_ = gi // ST, gi % ST
                sl, hp = SLHP[sidx]
                if tc_ == 0:
                    po_cur[0] = psPO.tile([65, 512], F32, tag="po", name=f"po0_{sidx}")
                    po_cur[1] = psPO.tile([65, 512], F32, tag="po", name=f"po1_{sidx}")
                h0, h1 = 2 * hp, 2 * hp + 1
                nc.tensor.matmul(
                    po_cur[0][:], v_sb[tc_][:, h0 * 65:(h0 + 1) * 65],
                    et[:, 0:512], start=(tc_ == 0), stop=(tc_ == ST - 1),
                )
                nc.tensor.matmul(
                    po_cur[1][:], v_sb[tc_][:, h1 * 65:(h1 + 1) * 65],
                    et[:, 512:1024], start=(tc_ == 0), stop=(tc_ == ST - 1),
                )

            def emit_norm(sidx, last):
                # NB: partition bases must be 32-aligned; the custom
                # reciprocal and the gpsimd broadcast both silently corrupt
                # on inputs not at partition 0 — so the den row is copied to
                # a [1,512] tile first.  Ops are interleaved across the two
                # halves so the gpsimd broadcasts run back to back.
                sl, hp = SLHP[sidx]
                srcs, rbcs = [], []
                for half in range(2):
                    po = po_cur[half]
                    drow = normp.tile([1, 512], F32, tag="drow", name="drow")
                    nc.vector.tensor_copy(drow[:], po[64:65, :])
                    if half == 0:
                        emit_norm.last_drow = drow
                    if last:
                        srcs.append(po[0:64, :])
                    else:
                        # copying the data rows out frees the PSUM bank for
                        # the next head-pair ~1.4us after the last PV
                        u = normp.tile([64, 512], F32, tag="u", name=f"u{sidx}{half}")
                        nc.vector.tensor_copy(u[:], po[0:64, :])
                        srcs.append(u[:])
                    rrow = normp.tile([1, 512], F32, tag="rrow", name="rrow")
                    nc.vector.reciprocal_approx_fast(rrow[:], drow[:])
                    rbc = normp.tile([64, 512], F32, tag="rbc", name="rbc")
                    nc.gpsimd.partition_broadcast(rbc[:], rrow[:])
                    rbcs.append(rbc)
                for half in range(2):
                    nc.vector.tensor_mul(
                        oT[hp][sl][half * 64:(half + 1) * 64, :],
                        srcs[half], rbcs[half][:],
                    )

            # ---------- filler schedule: (ready, deadline, thunk) ----------
            # deadline d = must be emitted during step d at the latest
            # (fillers pop before that step's PV).
            fillers = []

            def push(ready, deadline, thunks):
                for t in thunks:
                    fillers.append([ready, deadline, t])

            # V projections: v_sb[tc] consumed by PV at step tc.  QK groups
            # get staggered deadlines comfortably before their consumers
            # (scores for step 8*hp+s4 are emitted one step early) so no
            # single step carries a lump of forced pops.
            for t in range(4, ST):
                push(0, t, v_thunks(t))
            push(0, 1, qk_thunks(wkr, bk_sb, kT, 0, 1))         # d0 (<=2)
            push(0, 4, qk_thunks(wqr, bq_sb, qT, 1, 0))         # a1 (<=6)
            push(0, 5, qk_thunks(wkr, bk_sb, kT, 1, 0))         # c1 (<=6)
            push(0, 9, qk_thunks(wkr, bk_sb, kT, 1, 1))         # d1 (<=10)
            push(0, 11, qk_thunks(wqr, bq_sb, qT, 2, 0))        # a2 (<=14)
            push(0, 12, qk_thunks(wkr, bk_sb, kT, 2, 0))        # c2 (<=14)
            push(0, 17, qk_thunks(wkr, bk_sb, kT, 2, 1))        # d2 (<=18)
            push(0, 19, qk_thunks(wqr, bq_sb, qT, 3, 0))        # a3 (<=22)
            push(0, 20, qk_thunks(wkr, bk_sb, kT, 3, 0))        # c3 (<=22)
            push(0, 25, qk_thunks(wkr, bk_sb, kT, 3, 1))        # d3 (<=26)
            push(0, 28, qk_thunks(wqr, bq_sb, qT, 0, 1))        # b0 (<=30)
            push(0, 34, qk_thunks(wqr, bq_sb, qT, 1, 1))        # b1 (<=38)
            push(0, 42, qk_thunks(wqr, bq_sb, qT, 2, 1))        # b2 (<=46)
            push(0, 50, qk_thunks(wqr, bq_sb, qT, 3, 1))        # b3 (<=54)
            # out-proj slab0: oT[*][0] complete ~2 steps after the step-31
            # normalize is emitted.
            for ct in range(IT):
                push(34, (37, 41, 45, 49)[ct], op_thunks(ct))
            # out-proj slab1 partials: ic 0,1 land after the step-47
            # normalize, ic 2 after step-55.
            for ct in range(IT):
                push(50, 51 + ct, op_partial_a_thunks(ct))
            for ct in range(IT):
                push(58, 58 + ct, op_partial_b_thunks(ct))

            fillers.sort(key=lambda f: f[1])

            fq = list(fillers)

            def pop_fillers(gi):
                n = 0
                cap = 0 if gi % ST == ST - 1 else FILLER_CAP
                while fq and fq[0][0] <= gi and (fq[0][1] <= gi or n < cap):
                    fq.pop(0)[2]()
                    n += 1

            # ---------- emission ----------
            run = lambda ts: [t() for t in ts]
            for t in range(4):
                run(v_thunks(t))
            run(qk_thunks(wqr, bq_sb, qT, 0, 0))
            run(qk_thunks(wkr, bk_sb, kT, 0, 0))
            et_cur = emit_scores(0)
            for gi in range(NSTEP):
                et_next = emit_scores(gi + 1) if gi + 1 < NSTEP else None
                pop_fillers(gi)
                emit_pv(gi, et_cur)
                if gi % ST == ST - 1:
                    emit_norm(gi // ST, last=(gi == NSTEP - 1))
                et_cur = et_next
            assert not fq, f"{len(fq)} fillers left unscheduled"


            # ---------- tail: final out-proj (ic=3) for slab 1 ----------
            # two [128,1024] psum tiles from the (now idle) scores pool
            fins = [psS.tile([128, 1024], F32, tag="pss", name=f"fin{pair}")
                    for pair in range(2)]
            for half in range(2):
                for ct in range(IT):
                    nc.tensor.matmul(
                        fins[ct // 2][:, (ct % 2) * 512:(ct % 2 + 1) * 512],
                        wor[half * 64:(half + 1) * 64, IT - 1,
                            ct * 128:(ct + 1) * 128],
                        oT[IT - 1][1][half * 64:(half + 1) * 64, :],
                        start=(half == 0), stop=(half == 1),
                        tile_position=(64 * half, 0),
                    )
            for ct in range(IT):
                fin = fins[ct // 2]
                j = ct % 2
                ysb = outp.tile([128, 512], F16, tag="ysbf", name="ysbf",
                                bufs=4)
                nc.vector.tensor_add(ysb[:], y_part[:, ct, :],
                                     fin[:, j * 512:(j + 1) * 512])
                eng = nc.sync if ct % 2 == 0 else nc.gpsimd
                eng.dma_start(y_d[ct * 128:(ct + 1) * 128, 512:1024], ysb[:])

    nc.compile()
    return nc


def prep_host(inputs, dt_mm):
    """Fold BN + scale + v-bias into effective weights (fp32 numpy)."""
    x = np.asarray(inputs["x"], dtype=np.float32)
    g = np.asarray(inputs["bn_gamma"], dtype=np.float32)
    be = np.asarray(inputs["bn_beta"], dtype=np.float32)
    mu = np.asarray(inputs["bn_mean"], dtype=np.float32)
    var = np.asarray(inputs["bn_var"], dtype=np.float32)
    wq = np.asarray(inputs["wq"], dtype=np.float32)
    bq = np.asarray(inputs["bq"], dtype=np.float32)
    wk = np.asarray(inputs["wk"], dtype=np.float32)
    bk = np.asarray(inputs["bk"], dtype=np.float32)
    wv = np.asarray(inputs["wv"], dtype=np.float32)
    bv = np.asarray(inputs["bv"], dtype=np.float32)
    wo = np.asarray(inputs["wo"], dtype=np.float32)
    bo = np.asarray(inputs["bo"], dtype=np.float32)

    a = g / np.sqrt(var + EPS)          # [C]
    bvec = be - mu * a                  # [C]

    wq_eff = wq * a[None, :] * SCALE
    bq_eff = (bq + wq @ bvec) * SCALE
    wk_eff = wk * a[None, :]
    bk_eff = bk + wk @ bvec
    wv_eff = wv * a[None, :]
    bv_eff = bv + wv @ bvec
    bo_eff = bo + wo @ bv_eff           # v bias rides through softmax (sums to 1)

    bias_pack = np.concatenate(
        [bq_eff.reshape(IT, 128).T, bk_eff.reshape(IT, 128).T,
         bo_eff.reshape(IT, 128).T], axis=1
    ).astype(np.float32)

    np_dt = np.float16

    def dev_layout(a):
        # [C_or_I, N] -> [128, KC, N]: partition p holds rows {k*128+p}
        return np.ascontiguousarray(
            a.reshape(KC, 128, a.shape[1]).transpose(1, 0, 2).astype(np_dt))

    def dev_layout_hp(a):
        # [C, I] -> [128, IT, KC, 128]: [p, i, k, c] = a[k*128+p, i*128+c]
        return np.ascontiguousarray(
            a.reshape(KC, 128, IT, 128).transpose(1, 2, 0, 3).astype(np_dt))

    def dev_layout_x(a):
        # [C, S] -> [128, NSLAB, KC, 512]: [p, s, k, c] = a[k*128+p, s*512+c]
        return np.ascontiguousarray(
            a.reshape(KC, 128, NSLAB, 512).transpose(1, 2, 0, 3).astype(np_dt))

    wqk_l = np.ascontiguousarray(np.stack(
        [dev_layout_hp(wq_eff.T), dev_layout_hp(wk_eff.T)], axis=1))
    wv_l = dev_layout(wv_eff.T)
    wo_l = dev_layout(wo.T)
    per_core = []
    for b in range(B):
        per_core.append({
            "x": dev_layout_x(x[b, :, :, 0]),
            "wqkT": wqk_l,
            "wvT": wv_l,
            "woT": wo_l,
            "bias_pack": np.ascontiguousarray(bias_pack),
        })
    return per_core


def _get_nc(dt_mm):
    key = str(dt_mm)
    if key not in _CACHE:
        _CACHE[key] = build_bass(dt_mm)
    return _CACHE[key]


def kernel(**inputs):
    nc = _get_nc(DT_MM)
    in_maps = prep_host(inputs, DT_MM)
    res = run_bass_kernel_spmd(nc, in_maps, list(range(N_CORES)))
    y = np.stack([res.results[c]["y"].astype(np.float32)
                  for c in range(N_CORES)], axis=0)
    return y[..., None]


def run_traced(**inputs):
    """Like kernel() but with NTFF profiling; returns (y, results, tmpdir)."""
    nc = _get_nc(DT_MM)
    in_maps = prep_host(inputs, DT_MM)
    import tempfile
    tmpdir = tempfile.mkdtemp(prefix="mha_trace_")
    res = run_bass_kernel_spmd(
        nc, in_maps, list(range(N_CORES)), trace=True, tmpdir=tmpdir
    )
    y = np.stack([res.results[c]["y"].astype(np.float32)
                  for c in range(N_CORES)], axis=0)
    return y[..., None], res, tmpdir
